# revision 11
# baseline (speedup 1.0000x reference)
"""V3: reversed matmul formulation (folded moving operand becomes the
stationary; A slabs are the moving operand at N=512), full A residency in
SBUF (A read from HBM exactly once), scoped X^T pool so its 8MB is reused.

Math (same as V2): S = r*(U + U2) + nmr*deg x wbar + cvec, computed
transposed: big matmuls produce [coeffs, nodes] tiles that are
PE-transposed back to node-major for softmax / output.
"""

import os
import sys

sys.path.insert(0, "/opt/trn_rl_repo")
os.environ.setdefault("MYCRO_LOCAL_CACHE", "1")

import numpy as np
import ml_dtypes
from contextlib import ExitStack

import concourse.bass as bass  # noqa: F401
import concourse.bacc as bacc
import concourse.mybir as mybir
import concourse.tile as tile
import concourse.bass_isa as bass_isa
from concourse.bass_utils import run_bass_kernel_spmd

N, T, F, C = 8192, 256, 128, 32
NCORES = 8
ML = N // NCORES
KT = N // 128
MT = ML // 128
EPS = 1e-5
FP = mybir.dt.float32
BF = mybir.dt.bfloat16
NSUP = 12             # pinned A supertiles of 4 k-tiles each (k < 48)
NPIN = 4 * NSUP

_cache = {}
last_exec_time_ns = None


def _build(reps=1):
    nc = bacc.Bacc("TRN2", target_bir_lowering=False, debug=False,
                   num_devices=NCORES)

    Ash = nc.declare_dram_parameter("Ash", [N, ML], BF, isOutput=False)
    XT = nc.declare_dram_parameter("XT", [T, N], FP, isOutput=False)
    XlT = nc.declare_dram_parameter("XlT", [T, ML], FP, isOutput=False)
    Wrel = nc.declare_dram_parameter("Wrel", [F, T], FP, isOutput=False)
    Wroot = nc.declare_dram_parameter("Wroot", [F, T], FP, isOutput=False)
    WmT = nc.declare_dram_parameter("WmT", [F, C], FP, isOutput=False)
    brel = nc.declare_dram_parameter("brel", [F, 1], FP, isOutput=False)
    bmlp = nc.declare_dram_parameter("bmlp", [C, 1], FP, isOutput=False)
    I128 = nc.declare_dram_parameter("I128", [128, 128], FP, isOutput=False)
    Sout = nc.declare_dram_parameter("S_out", [ML, C], FP, isOutput=True)
    Lout = nc.declare_dram_parameter("losses", [1, 2], FP, isOutput=True)

    with tile.TileContext(nc) as tc:
        for rep in range(reps):
            _body(nc, tc, rep, Ash, XT, XlT, Wrel, Wroot, WmT, brel, bmlp,
                  I128, Sout, Lout)

    nc.compile()
    return nc


def _body(nc, tc, rep, Ash, XT, XlT, Wrel, Wroot, WmT, brel, bmlp, I128,
          Sout, Lout):
    fx = mybir.ActivationFunctionType
    alu = mybir.AluOpType

    ctx = ExitStack()
    sb = ctx.enter_context(tc.tile_pool(name=f"sb{rep}", bufs=1))
    dram = ctx.enter_context(tc.tile_pool(name=f"dram{rep}", bufs=1,
                                          space="DRAM"))
    ps = ctx.enter_context(tc.tile_pool(name=f"ps{rep}", bufs=1,
                                        space="PSUM"))
    a1 = ctx.enter_context(tc.tile_pool(name=f"a1_{rep}", bufs=1))

    # ---------- weights ----------
    ident = sb.tile([128, 128], FP, name="ident", tag="ident")
    nc.sync.dma_start(ident[:, :], I128[:, :])
    wrel_sb = sb.tile([F, T], FP, name="wrel_sb", tag="wrel_sb")
    wroot_sb = sb.tile([F, T], FP, name="wroot_sb", tag="wroot_sb")
    wm = sb.tile([F, C], FP, name="wm", tag="wm")
    brel_sb = sb.tile([F, 1], FP, name="brel_sb", tag="brel_sb")
    bmlp_sb = sb.tile([C, 1], FP, name="bmlp_sb", tag="bmlp_sb")
    nc.sync.dma_start(wrel_sb[:, :], Wrel[:, :])
    nc.sync.dma_start(wroot_sb[:, :], Wroot[:, :])
    nc.sync.dma_start(wm[:, :], WmT[:, :])
    nc.sync.dma_start(brel_sb[:, :], brel[:, :])
    nc.sync.dma_start(bmlp_sb[:, :], bmlp[:, :])
    ones128 = sb.tile([128, 1], FP, name="ones128", tag="ones128")
    nc.gpsimd.memset(ones128[:, :], 1.0)

    wr2T = [sb.tile([128, C], FP, name=f"wr2T{t}", tag=f"wr2T{t}")
            for t in range(2)]
    wq2T = [sb.tile([128, C], FP, name=f"wq2T{t}", tag=f"wq2T{t}")
            for t in range(2)]
    for t in range(2):
        pw = ps.tile([128, C], FP, name="pw", tag="pk", bufs=3)
        nc.tensor.matmul(pw[:, :], wrel_sb[:, t * 128:(t + 1) * 128],
                         wm[:, :], start=True, stop=True)
        nc.vector.tensor_copy(wr2T[t][:, :], pw[:, :])
        pw2 = ps.tile([128, C], FP, name="pw2", tag="pk", bufs=3)
        nc.tensor.matmul(pw2[:, :], wroot_sb[:, t * 128:(t + 1) * 128],
                         wm[:, :], start=True, stop=True)
        nc.vector.tensor_copy(wq2T[t][:, :], pw2[:, :])
    pb = ps.tile([C, 1], FP, name="pb", tag="pk", bufs=3)
    nc.tensor.matmul(pb[:, :], wm[:, :], brel_sb[:, :], start=True, stop=True)
    b2c = sb.tile([C, 1], FP, name="b2c", tag="b2c")
    nc.vector.tensor_add(b2c[:, :], pb[:, :], bmlp_sb[:, :])
    pbr = ps.tile([1, C], FP, name="pbr", tag="pk", bufs=3)
    nc.tensor.transpose(pbr[:, :], b2c[:, :], ident[0:C, 0:C])
    b2row = sb.tile([1, C], FP, name="b2row", tag="b2row")
    nc.vector.tensor_copy(b2row[:, :], pbr[:, :])
    b2b = sb.tile([128, C], FP, name="b2b", tag="b2b")
    nc.gpsimd.partition_broadcast(b2b[:, :], b2row[:, :])
    pwb = ps.tile([1, C], FP, name="pwb", tag="pk", bufs=3)
    for t in range(2):
        nc.tensor.matmul(pwb[:, :], ones128[:, :], wr2T[t][:, :],
                         start=(t == 0), stop=(t == 1))
    wbar1 = sb.tile([1, C], FP, name="wbar1", tag="wbar1")
    nc.vector.tensor_copy(wbar1[:, :], pwb[:, :])
    wbar = sb.tile([128, C], FP, name="wbar", tag="wbar")
    nc.gpsimd.partition_broadcast(wbar[:, :], wbar1[:, :])
    pqb = ps.tile([1, C], FP, name="pqb", tag="pk", bufs=3)
    for t in range(2):
        nc.tensor.matmul(pqb[:, :], ones128[:, :], wq2T[t][:, :],
                         start=(t == 0), stop=(t == 1))
    qbar1 = sb.tile([1, C], FP, name="qbar1", tag="qbar1")
    nc.vector.tensor_copy(qbar1[:, :], pqb[:, :])
    qbar = sb.tile([128, C], FP, name="qbar", tag="qbar")
    nc.gpsimd.partition_broadcast(qbar[:, :], qbar1[:, :])

    # ---------- A supertiles + streamed X^T chunks; zmov production ----
    W1 = 2 * C + 1  # 65
    zmov = [sb.tile([128, 97], BF, name=f"zs{k}", tag=f"zs{k}")
            for k in range(KT)]
    NCHK = 8
    CW = N // NCHK  # 1024
    pstat = sb.tile([128, 2 * NCHK], FP, name="pstat", tag="pstat")
    pstat2 = sb.tile([128, 2 * NCHK], FP, name="pstat2", tag="pstat2")
    asup = {}
    astr = {}

    def a_moving(k, chn, tab):
        if k < NPIN:
            return asup[k // 4][:, k % 4, chn * 512:(chn + 1) * 512]
        return tab[k][:, chn * 512:(chn + 1) * 512]

    ash4 = Ash.ap().rearrange("(g p) m -> p g m", p=128)
    for c in range(NCHK):
        for sc in (2 * c, 2 * c + 1):
            if sc < NSUP:
                asup[sc] = a1.tile([128, 4, ML], BF, name=f"asup{sc}",
                                   tag=f"asup{sc}")
                nc.sync.dma_start(asup[sc][:, :, :],
                                  ash4[:, 4 * sc:4 * (sc + 1), :])
            elif sc == NSUP:
                for k in range(NPIN, KT):
                    astr[k] = sb.tile([128, ML], BF, name=f"astr{k}",
                                      tag="astr", bufs=4)
                    nc.sync.dma_start(astr[k][:, :],
                                      Ash[k * 128:(k + 1) * 128, :])
        xtc = [sb.tile([128, CW], FP, name=f"xtc{t}", tag=f"xtc{t}", bufs=2)
               for t in range(2)]
        for t in range(2):
            nc.sync.dma_start(xtc[t][:, :],
                              XT[t * 128:(t + 1) * 128,
                                 c * CW:(c + 1) * CW])
            col = t * NCHK + c
            nc.vector.tensor_reduce(pstat[:, col:col + 1], xtc[t][:, :],
                                    mybir.AxisListType.X, alu.add)
            sq = sb.tile([128, CW], FP, name="sq", tag="sq", bufs=2)
            nc.scalar.activation(sq[:, :], xtc[t][:, :],
                                 fx.Square, accum_out=pstat2[:, col:col + 1])
        for kk in range(8):
            k = c * 8 + kk
            px = ps.tile([128, C], FP, name="px", tag="pk", bufs=3)
            for t in range(2):
                nc.tensor.matmul(px[:, :],
                                 xtc[t][:, kk * 128:(kk + 1) * 128],
                                 wr2T[t][:, :], start=(t == 0), stop=(t == 1))
            nc.gpsimd.memset(zmov[k][:, 2 * C:2 * C + 1], 1.0)
            nc.vector.tensor_copy(zmov[k][:, 0:C], px[:, :])
            hif = sb.tile([128, C], FP, name="hif", tag="hif", bufs=3)
            nc.scalar.copy(hif[:, :], zmov[k][:, 0:C])
            lof = sb.tile([128, C], FP, name="lof", tag="lof", bufs=3)
            nc.vector.tensor_sub(lof[:, :], px[:, :], hif[:, :])
            nc.vector.tensor_copy(zmov[k][:, C:2 * C], lof[:, :])

    # ---------- stats -> r, nmr, cvec ----------
    st2 = sb.tile([128, 2], FP, name="st2", tag="st2")
    nc.vector.tensor_reduce(st2[:, 0:1], pstat[:, :], mybir.AxisListType.X,
                            alu.add)
    nc.vector.tensor_reduce(st2[:, 1:2], pstat2[:, :], mybir.AxisListType.X,
                            alu.add)
    stall = sb.tile([128, 2], FP, name="stall", tag="stall")
    nc.gpsimd.partition_all_reduce(stall[:, :], st2[:, :], channels=128,
                                   reduce_op=bass_isa.ReduceOp.add)
    mu = sb.tile([128, 1], FP, name="mu", tag="mu")
    ex2 = sb.tile([128, 1], FP, name="ex2", tag="ex2")
    nc.scalar.mul(mu[:, :], stall[:, 0:1], 1.0 / (N * T))
    nc.scalar.mul(ex2[:, :], stall[:, 1:2], 1.0 / (N * T))
    mu2 = sb.tile([128, 1], FP, name="mu2", tag="mu2")
    nc.scalar.activation(mu2[:, :], mu[:, :], fx.Square)
    var = sb.tile([128, 1], FP, name="var", tag="var")
    nc.vector.tensor_sub(var[:, :], ex2[:, :], mu2[:, :])
    nc.vector.tensor_scalar_add(var[:, :], var[:, :], EPS)
    sd = sb.tile([128, 1], FP, name="sd", tag="sd")
    nc.scalar.activation(sd[:, :], var[:, :], fx.Sqrt)
    rr = sb.tile([128, 1], FP, name="rr", tag="rr")
    nc.vector.reciprocal(rr[:, :], sd[:, :])
    nmr = sb.tile([128, 1], FP, name="nmr", tag="nmr")
    nc.vector.tensor_mul(nmr[:, :], mu[:, :], rr[:, :])
    nc.vector.tensor_scalar_mul(nmr[:, :], nmr[:, :], -1.0)
    cvec = sb.tile([128, C], FP, name="cvec", tag="cvec")
    nc.vector.tensor_scalar_mul(cvec[:, :], qbar[:, :], nmr[:, 0:1])
    nc.vector.tensor_add(cvec[:, :], cvec[:, :], b2b[:, :])

    # ---------- U2^T = Wq2T.T @ XlocT  [32, 1024] ----------
    xlT = [sb.tile([128, ML], FP, name=f"xlT{t}", tag=f"xlT{t}")
           for t in range(2)]
    for t in range(2):
        nc.sync.dma_start(xlT[t][:, :], XlT[t * 128:(t + 1) * 128, :])
    u2T = sb.tile([C, ML], FP, name="u2T", tag="u2T")
    for ch in range(2):
        pu = ps.tile([C, 512], FP, name="pu", tag="pk", bufs=3)
        for t in range(2):
            nc.tensor.matmul(pu[:, :], wq2T[t][:, :],
                             xlT[t][:, ch * 512:(ch + 1) * 512],
                             start=(t == 0), stop=(t == 1))
        nc.vector.tensor_copy(u2T[:, ch * 512:(ch + 1) * 512], pu[:, :])

    # ---------- phase 1 (reversed): [U^T; deg^T] = zmov.T @ A ----------
    ps1r = [ps.tile([W1, 512], FP, name=f"ps1r{i}", tag=f"p1_{i}", bufs=1)
            for i in range(2)]
    for k in range(KT):
        for chn in range(2):
            nc.tensor.matmul(ps1r[chn][:, :], zmov[k][:, 0:W1],
                             a_moving(k, chn, astr),
                             start=(k == 0), stop=(k == KT - 1))
    # evict: utd [33, 1024] = [U_hi+U_lo+U2 ; deg]
    utd = sb.tile([C + 1, ML], FP, name="utd", tag="utd")
    for chn in range(2):
        sl = slice(chn * 512, (chn + 1) * 512)
        nc.vector.tensor_copy(utd[0:C, sl], ps1r[chn][0:C, :])
        nc.vector.tensor_add(utd[0:C, sl], utd[0:C, sl],
                             ps1r[chn][C:2 * C, :])
        nc.vector.tensor_copy(utd[C:C + 1, sl], ps1r[chn][2 * C:2 * C + 1, :])
    nc.vector.tensor_add(utd[0:C, :], utd[0:C, :], u2T[:, :])

    # ---------- per m-tile: transpose, S assembly, softmax, z ----------
    dinv = sb.tile([128, MT], FP, name="dinv", tag="dinv")
    dsq = sb.tile([128, MT], FP, name="dsq", tag="dsq")
    svals = [sb.tile([128, C], FP, name=f"svals{m}", tag=f"svals{m}")
             for m in range(MT)]
    ssm = [sb.tile([128, C], FP, name=f"ssm{m}", tag=f"ssm{m}")
           for m in range(MT)]
    cc_in1 = dram.tile([ML, C + 1], FP, name="cc_in1", tag="cc_in1")
    cc_out1 = dram.tile([N, C + 1], FP, name="cc_out1", tag="cc_out1",
                        addr_space="Shared")
    for m in range(MT):
        stp = ps.tile([128, C + 1], FP, name="stp", tag="tp", bufs=3)
        nc.tensor.transpose(stp[:, :], utd[:, m * 128:(m + 1) * 128],
                            ident[0:C + 1, 0:C + 1])
        # deg -> dinv
        nc.scalar.activation(dsq[:, m:m + 1], stp[:, C:C + 1], fx.Sqrt)
        nc.vector.reciprocal(dinv[:, m:m + 1], dsq[:, m:m + 1])
        # S = r*(U+U2) + wbar*(nmr*deg) + cvec
        su = svals[m]
        nc.vector.tensor_scalar_mul(su[:, :], stp[:, 0:C], rr[:, 0:1])
        rmd = sb.tile([128, 1], FP, name="rmd", tag="rmd", bufs=2)
        nc.vector.tensor_mul(rmd[:, :], stp[:, C:C + 1], nmr[:, :])
        t2 = sb.tile([128, C], FP, name="t2", tag="t2", bufs=2)
        nc.vector.tensor_scalar_mul(t2[:, :], wbar[:, :], rmd[:, 0:1])
        nc.vector.tensor_add(su[:, :], su[:, :], t2[:, :])
        nc.vector.tensor_add(su[:, :], su[:, :], cvec[:, :])
        nc.sync.dma_start(Sout[m * 128:(m + 1) * 128, :], su[:, :])
        # softmax
        nmx = sb.tile([128, 1], FP, name="nmx", tag="nmx", bufs=2)
        nc.vector.tensor_reduce(nmx[:, :], su[:, :], mybir.AxisListType.X,
                                alu.max, negate=True)
        esum = sb.tile([128, 1], FP, name="esum", tag="esum", bufs=2)
        nc.scalar.activation(ssm[m][:, :], su[:, :], fx.Exp,
                             bias=nmx[:, 0:1], accum_out=esum[:, :])
        rsum = sb.tile([128, 1], FP, name="rsum", tag="rsum", bufs=2)
        nc.vector.reciprocal(rsum[:, :], esum[:, :])
        nc.vector.tensor_scalar_mul(ssm[m][:, :], ssm[m][:, :], rsum[:, 0:1])
        z = sb.tile([128, C + 1], FP, name="z", tag="z", bufs=4)
        nc.vector.tensor_scalar_mul(z[:, 0:C], ssm[m][:, :], dinv[:, m:m + 1])
        nc.vector.tensor_copy(z[:, C:C + 1], dinv[:, m:m + 1])
        nc.sync.dma_start(cc_in1[m * 128:(m + 1) * 128, :], z[:, :])

    nc.gpsimd.collective_compute(
        "AllGather", alu.bypass,
        replica_groups=[list(range(NCORES))],
        ins=[cc_in1.opt()], outs=[cc_out1.opt()],
    )

    # ---------- zm2 [hi(0:33) | junk(33:64) | lo(64:97)] ----------
    W2 = 97
    zt3 = sb.tile([128, KT, C + 1], FP, name="zt3", tag="zt3")
    cc1v = cc_out1.tensor.ap().rearrange("(k p) c -> p k c", p=128)
    nc.sync.dma_start(zt3[:, :, :], cc1v[:, :, :])
    for k in range(KT):
        nc.vector.tensor_copy(zmov[k][:, 0:C + 1], zt3[:, k, :])
        hif2 = sb.tile([128, C + 1], FP, name="hif2", tag="hif", bufs=3)
        nc.scalar.copy(hif2[:, :], zmov[k][:, 0:C + 1])
        lof2 = sb.tile([128, C + 1], FP, name="lof2", tag="lof", bufs=3)
        nc.vector.tensor_sub(lof2[:, :], zt3[:, k, :], hif2[:, :])
        nc.vector.tensor_copy(zmov[k][:, 2 * C:3 * C + 1], lof2[:, :])

    # ---------- phase 2 (reversed) ----------
    ps2r = [ps.tile([W2, 512], FP, name=f"ps2r{i}", tag=f"p1_{i}", bufs=1)
            for i in range(2)]
    astr2 = {}
    for k in range(NPIN, KT):
        astr2[k] = sb.tile([128, ML], BF, name=f"astr2_{k}", tag="astr2",
                           bufs=4)
        nc.sync.dma_start(astr2[k][:, :], Ash[k * 128:(k + 1) * 128, :])
    for k in range(KT):
        for chn in range(2):
            nc.tensor.matmul(ps2r[chn][:, :], zmov[k][:, 0:W2],
                             a_moving(k, chn, astr2),
                             start=(k == 0), stop=(k == KT - 1))
    # evict: w [33, 1024] = [U'_hi+U'_lo ; Ad_hi+Ad_lo]
    w_sb = sb.tile([C + 1, ML], FP, name="w_sb", tag="utd")
    for chn in range(2):
        sl = slice(chn * 512, (chn + 1) * 512)
        nc.vector.tensor_copy(w_sb[0:C, sl], ps2r[chn][0:C, :])
        nc.vector.tensor_add(w_sb[0:C, sl], w_sb[0:C, sl],
                             ps2r[chn][2 * C:3 * C, :])
        nc.vector.tensor_copy(w_sb[C:C + 1, sl], ps2r[chn][C:C + 1, :])
        nc.vector.tensor_add(w_sb[C:C + 1, sl], w_sb[C:C + 1, sl],
                             ps2r[chn][3 * C:3 * C + 1, :])

    asx = [sb.tile([128, 3 * C + 1], FP, name=f"asx{m}", tag=f"asx{m}")
           for m in range(MT)]
    for m in range(MT):
        wtp = ps.tile([128, C + 1], FP, name="wtp", tag="tp", bufs=3)
        nc.tensor.transpose(wtp[:, :], w_sb[:, m * 128:(m + 1) * 128],
                            ident[0:C + 1, 0:C + 1])
        nc.vector.tensor_scalar_mul(asx[m][:, 0:C + 1], wtp[:, :],
                                    dinv[:, m:m + 1])
        nc.vector.tensor_copy(asx[m][:, C + 1:2 * C + 1], ssm[m][:, :])
        nc.vector.tensor_scalar_mul(asx[m][:, 2 * C + 1:3 * C + 1],
                                    ssm[m][:, :], asx[m][:, C:C + 1])

    # ---------- partials + all-reduce + losses ----------
    i32 = sb.tile([C, C], FP, name="i32", tag="i32")
    nc.vector.tensor_copy(i32[:, :], ident[0:C, 0:C])
    pack = sb.tile([C, 2 * C + 1], FP, name="pack", tag="pack")
    pp = ps.tile([C, 3 * C + 1], FP, name="ppart", tag="pk", bufs=3)
    for m in range(MT):
        nc.tensor.matmul(pp[:, :], ssm[m][:, :], asx[m][:, :],
                         start=(m == 0), stop=(m == MT - 1))
    nc.vector.tensor_copy(pack[:, 0:C], pp[:, 0:C])
    nc.vector.tensor_copy(pack[:, C:2 * C], pp[:, C + 1:2 * C + 1])
    tds = sb.tile([C, C], FP, name="tds", tag="tds")
    nc.vector.tensor_copy(tds[:, :], pp[:, 2 * C + 1:3 * C + 1])
    nc.vector.tensor_mul(tds[:, :], tds[:, :], i32[:, :])
    denp = sb.tile([C, 1], FP, name="denp", tag="denp")
    nc.vector.tensor_reduce(denp[:, :], tds[:, :], mybir.AxisListType.X,
                            alu.add)
    denall = sb.tile([C, 1], FP, name="denall", tag="denall")
    nc.gpsimd.partition_all_reduce(denall[:, :], denp[:, :], channels=C,
                                   reduce_op=bass_isa.ReduceOp.add)
    nc.vector.tensor_copy(pack[:, 2 * C:2 * C + 1], denall[:, :])

    cc_in2 = dram.tile([C, 2 * C + 1], FP, name="cc_in2", tag="cc_in2")
    cc_out2 = dram.tile([C, 2 * C + 1], FP, name="cc_out2", tag="cc_out2",
                        addr_space="Shared")
    nc.sync.dma_start(cc_in2[:, :], pack[:, :])
    nc.gpsimd.collective_compute(
        "AllReduce", alu.add,
        replica_groups=[list(range(NCORES))],
        ins=[cc_in2.opt()], outs=[cc_out2.opt()],
    )
    red = sb.tile([C, 2 * C + 1], FP, name="red", tag="red")
    nc.sync.dma_start(red[:, :], cc_out2[:, :])

    t1 = sb.tile([C, C], FP, name="t1", tag="t1")
    nc.vector.tensor_mul(t1[:, :], red[:, 0:C], i32[:, :])
    diag = sb.tile([C, 1], FP, name="diag", tag="diag")
    nc.vector.tensor_reduce(diag[:, :], t1[:, :], mybir.AxisListType.X,
                            alu.add)
    num32 = sb.tile([C, 1], FP, name="num32", tag="num32")
    nc.gpsimd.partition_all_reduce(num32[:, :], diag[:, :], channels=C,
                                   reduce_op=bass_isa.ReduceOp.add)
    recden = sb.tile([C, 1], FP, name="recden", tag="recden")
    nc.vector.reciprocal(recden[:, :], red[:, 2 * C:2 * C + 1])
    lm = sb.tile([C, 1], FP, name="lm", tag="lm")
    nc.vector.tensor_mul(lm[:, :], num32[:, :], recden[:, :])
    nc.vector.tensor_scalar_mul(lm[:, :], lm[:, :], -1.0)
    ssq = sb.tile([C, C], FP, name="ssq", tag="ssq")
    rowsq = sb.tile([C, 1], FP, name="rowsq", tag="rowsq")
    nc.scalar.activation(ssq[:, :], red[:, C:2 * C], fx.Square,
                         accum_out=rowsq[:, :])
    fro2 = sb.tile([C, 1], FP, name="fro2", tag="fro2")
    nc.gpsimd.partition_all_reduce(fro2[:, :], rowsq[:, :], channels=C,
                                   reduce_op=bass_isa.ReduceOp.add)
    fro = sb.tile([C, 1], FP, name="fro", tag="fro")
    nc.scalar.activation(fro[:, :], fro2[:, :], fx.Sqrt)
    rf = sb.tile([C, 1], FP, name="rf", tag="rf")
    nc.vector.reciprocal(rf[:, :], fro[:, :])
    en = sb.tile([C, C], FP, name="en", tag="en")
    nc.vector.tensor_scalar_mul(en[:, :], red[:, C:2 * C], rf[:, 0:1])
    i2 = sb.tile([C, C], FP, name="i2", tag="i2")
    nc.scalar.mul(i2[:, :], i32[:, :], 1.0 / float(np.sqrt(C)))
    nc.vector.tensor_sub(en[:, :], en[:, :], i2[:, :])
    e2 = sb.tile([C, C], FP, name="e2", tag="e2")
    e2r = sb.tile([C, 1], FP, name="e2r", tag="e2r")
    nc.scalar.activation(e2[:, :], en[:, :], fx.Square, accum_out=e2r[:, :])
    lo2 = sb.tile([C, 1], FP, name="lo2", tag="lo2")
    nc.gpsimd.partition_all_reduce(lo2[:, :], e2r[:, :], channels=C,
                                   reduce_op=bass_isa.ReduceOp.add)
    lo = sb.tile([C, 1], FP, name="lo", tag="lo")
    nc.scalar.activation(lo[:, :], lo2[:, :], fx.Sqrt)

    lout = sb.tile([1, 2], FP, name="lout", tag="lout")
    nc.vector.tensor_copy(lout[0:1, 0:1], lm[0:1, 0:1])
    nc.vector.tensor_copy(lout[0:1, 1:2], lo[0:1, 0:1])
    nc.sync.dma_start(Lout[:, :], lout[:, :])

    ctx.close()


def _prep_inputs(X, A, W_rel, b_rel, W_root, W_mlp, b_mlp):
    X = np.ascontiguousarray(np.asarray(X, dtype=np.float32))
    A = np.asarray(A, dtype=np.float32)
    XT_np = np.ascontiguousarray(X.T)
    Wrel_np = np.ascontiguousarray(np.asarray(W_rel, np.float32))
    Wroot_np = np.ascontiguousarray(np.asarray(W_root, np.float32))
    WmT_np = np.ascontiguousarray(np.asarray(W_mlp, np.float32).T)
    brel_np = np.ascontiguousarray(np.asarray(b_rel, np.float32).reshape(F, 1))
    bmlp_np = np.ascontiguousarray(np.asarray(b_mlp, np.float32).reshape(C, 1))
    I128_np = np.eye(128, dtype=np.float32)
    in_maps = []
    for c in range(NCORES):
        cols = slice(c * ML, (c + 1) * ML)
        in_maps.append({
            "Ash": np.ascontiguousarray(A[:, cols]).astype(ml_dtypes.bfloat16),
            "XT": XT_np,
            "XlT": np.ascontiguousarray(XT_np[:, cols]),
            "Wrel": Wrel_np, "Wroot": Wroot_np, "WmT": WmT_np,
            "brel": brel_np, "bmlp": bmlp_np, "I128": I128_np,
        })
    return in_maps


def kernel(X, A, W_rel, b_rel, W_root, W_mlp, b_mlp):
    global last_exec_time_ns
    if "nc" not in _cache:
        _cache["nc"] = _build()
    nc = _cache["nc"]
    in_maps = _prep_inputs(X, A, W_rel, b_rel, W_root, W_mlp, b_mlp)
    trace = os.environ.get("BENCH_TRACE", "0") == "1"
    res = run_bass_kernel_spmd(nc, in_maps, list(range(NCORES)), trace=trace)
    last_exec_time_ns = res.exec_time_ns
    _cache["last_res"] = res
    S = np.concatenate([res.results[c]["S_out"] for c in range(NCORES)], axis=0)
    losses = res.results[0]["losses"]
    return (S.astype(np.float32),
            np.float32(losses[0, 0]),
            np.float32(losses[0, 1]))
